# revision 60
# baseline (speedup 1.0000x reference)
"""Channel-attention (XCA-style) kernel for TRN2, 8 NeuronCores, data-parallel
over batch (1 image per core).

Per image:
  q = conv3x3(y, Wq')    folded 1x1+3x3, fp8 DoubleRow matmuls
  k = conv3x3(x, Wk')    folded 1x1 + depthwise-3x3 (rank-1 taps), fp8 DR
  v = dw3x3(conv1x1(x))  1x1 on PE fp16, depthwise stencil on DVE/ACT/Pool
  G[c,d]   = sum_n q[c,n] k[d,n]
  S        = G * t[c] / (|q_c||k_d|)   (block-diagonal per head)
  A        = softmax_d(S)
  out      = (P @ A) @ v  via C^T = A @ P^T on device

q,k fp8 per-output-channel scaling cancels in the L2 normalization.
v kept fully in SBUF (no DRAM spill).
"""
import numpy as np

import concourse.bass as bass
import concourse.bacc as bacc
import concourse.mybir as mybir
import concourse.tile as tile
from concourse.masks import make_identity

F32 = mybir.dt.float32
FP16 = mybir.dt.float16
FP8 = mybir.dt.float8e4
DR = mybir.MatmulPerfMode.DoubleRow

B, C, H, W = 8, 192, 128, 128
HEADS = 8
CH = C // HEADS            # 24
N = H * W                  # 16384
WP = W + 4                 # padded row stride for y/x fp8 planes (132)
HP = H + 2                 # padded rows (130)
NG = 8                     # row groups
GR = H // NG               # rows per group (16)
RT = 4                     # rows per conv tile
NT = H // RT               # conv tiles (32)

OC = [(0, 128), (128, 64)]  # output-channel chunking (PSUM partition chunks)
CC = [(0, 96), (96, 96)]    # contraction chunking for fp16 1x1 convs
PL = (HP + 2) * WP          # padded plane size per channel (17424)
GROWS = 18                  # padded rows resident per group
VS = W + 1                  # v1 padded row stride (129)

# DW tap order: tap = (dy+1)*3 + (dx+1); tap 4 = center (stencil init)
TAPS = [(dy, dx) for dy in (-1, 0, 1) for dx in (-1, 0, 1)]
# stencil: mul engine per tap ('a' ACT, 'd' DVE); leaf-pair adds
# (off the acc chain) with their engine; remaining taps add into acc on DVE
MUL_ENG = {0: 'a', 1: 'd', 2: 'a', 3: 'd', 5: 'a', 6: 'd', 7: 'd', 8: 'a'}
LEAF_PAIRS = [((0, 1), 'p'), ((2, 3), 'p'), ((5, 6), 'p'), ((7, 8), 'd')]


def build(repeat=1, use_for_i=False,
          parts=("q", "k", "v", "dw", "gram", "attn", "fin")):
    nc = bacc.Bacc()
    tok_in = nc.dram_tensor("tok_in", [128, 16], F32, kind="ExternalInput")
    d_y8 = nc.dram_tensor("ypad8", [96, 2 * PL], FP8, kind="ExternalInput")
    d_x8 = nc.dram_tensor("xpad8", [96, 2 * PL], FP8, kind="ExternalInput")
    d_x = nc.dram_tensor("x16", [C, N], FP16, kind="ExternalInput")
    d_wq = nc.dram_tensor("wq8", [96, 9 * 2 * C], FP8, kind="ExternalInput")
    d_wk = nc.dram_tensor("wk8", [96, 9 * 2 * C], FP8, kind="ExternalInput")
    d_wv = nc.dram_tensor("wv", [C, C], FP16, kind="ExternalInput")
    d_wdw = nc.dram_tensor("wdwv", [C, 16], F32, kind="ExternalInput")
    d_wp = nc.dram_tensor("wproj", [C, C], FP16, kind="ExternalInput")
    d_tv = nc.dram_tensor("tvec", [C, 16], F32, kind="ExternalInput")
    d_msk = nc.dram_tensor("smask", [C, C], F32, kind="ExternalInput")
    d_out = nc.dram_tensor("out", [C, N], FP16, kind="ExternalOutput")
    d_tok = nc.dram_tensor("tok_out", [128, 16], F32, kind="ExternalOutput")

    with tile.TileContext(nc) as tc:
        with (
            tc.tile_pool(name="wp", bufs=1) as wp,
            tc.tile_pool(name="io", bufs=2) as io,
            tc.tile_pool(name="qt", bufs=1) as qtp,
            tc.tile_pool(name="dw", bufs=2) as dwp,
            tc.tile_pool(name="sm", bufs=1) as sm,
            tc.tile_pool(name="ps", bufs=1, space="PSUM") as ps,
        ):
            tki = sm.tile([128, 16], F32)
            nc.sync.dma_start(tki, tok_in[:, :])

            t_wq8 = wp.tile([96, 9 * 2 * C], FP8, name="wq8")
            nc.sync.dma_start(t_wq8, d_wq[:, :])
            t_wk8 = wp.tile([96, 9 * 2 * C], FP8, name="wk8")
            nc.sync.dma_start(t_wk8, d_wk[:, :])
            t_wv = {}
            for c0, cn in CC:
                t_wv[c0] = wp.tile([cn, C], FP16, name=f"wv{c0}")
                nc.sync.dma_start(t_wv[c0], d_wv[c0:c0 + cn, :])
            t_wp_ = {}
            t_tv = {}
            t_msk = {}
            t_wdw = {}
            for c0, cn in OC:
                t_wp_[c0] = wp.tile([cn, C], FP16, name=f"wp{c0}")
                nc.sync.dma_start(t_wp_[c0], d_wp[c0:c0 + cn, :])
                t_tv[c0] = wp.tile([cn, 16], F32, name=f"tv{c0}")
                nc.sync.dma_start(t_tv[c0], d_tv[c0:c0 + cn, :])
                t_msk[c0] = wp.tile([cn, C], F32, name=f"msk{c0}")
                nc.sync.dma_start(t_msk[c0], d_msk[c0:c0 + cn, :])
                t_wdw[c0] = wp.tile([cn, 16], F32, name=f"wdw{c0}")
                nc.sync.dma_start(t_wdw[c0], d_wdw[c0:c0 + cn, :])
            id16 = wp.tile([128, 128], FP16, name="id16")
            make_identity(nc, id16)
            id32 = wp.tile([128, 128], F32, name="id32")
            make_identity(nc, id32)
            ones1 = wp.tile([1, C], F32, name="ones1")
            nc.vector.memset(ones1, 1.0)

            state = {}

            def body(it=None):
                t_qT = qtp.tile([128, H * C], FP16, name="qT", tag="qT")
                qT3 = t_qT[:, :].rearrange("p (j c) -> p j c", c=C)
                v_sb = {0: qtp.tile([128, N], FP16, name="vsbA", tag="vsbA"),
                        128: qtp.tile([64, N], FP16, name="vsbB", tag="vsbB")}
                pGall = ps.tile([128, 2 * C], F32, name="pGall", tag="pGall")
                pG = {0: pGall[:, 0:C], 128: pGall[0:64, C:2 * C]}
                qn = {o0: sm.tile([on, 32], F32, name=f"qn{o0}",
                            tag=f"qn{o0}") for o0, on in OC}
                kn = {o0: sm.tile([on, 32], F32, name=f"kn{o0}",
                            tag=f"kn{o0}") for o0, on in OC}
                v1 = {}

                def fp8_conv_tile(t, d_src, t_w, pq_tags, stage_pfx,
                                  kT_g=None):
                    """3x3 fp8-DR conv tile: 18 matmuls, copies, norms,
                    transposes. Writes qT (stage_pfx 'q') or kT_g ('k')."""
                    g = (RT * t) // GR
                    r0 = RT * t
                    ykey = stage_pfx + "src"
                    if r0 % GR == 0:
                        ty = io.tile([96, 2 * GROWS * WP], FP8,
                                     name=stage_pfx + "8", tag=stage_pfx + "8",
                                     bufs=2)
                        for j in range(2):
                            nc.sync.dma_start(
                                ty[:, j * GROWS * WP:(j + 1) * GROWS * WP],
                                d_src[0:96, j * PL + (16 * g + 1) * WP:
                                      j * PL + (16 * g + 1 + GROWS) * WP])
                        state[ykey] = ty
                    ty = state[ykey]
                    psc = state["psc"]
                    pq = {0: psc.tile([128, 512], F32, name=pq_tags[0],
                                      tag="pq0", bufs=2),
                          128: psc.tile([64, 512], F32, name=pq_tags[1],
                                        tag="pq1", bufs=1)}
                    for o0, on in OC:
                        for tap in range(9):
                            dy, dx = TAPS[tap]
                            lr = r0 - 16 * g + 1 + dy
                            lhsT = bass.AP(
                                tensor=t_w.tensor,
                                offset=t_w.offset + tap * 2 * C + o0,
                                ap=[t_w.ap[0], [C, 2], [1, on]])
                            rhs = bass.AP(
                                tensor=ty.tensor,
                                offset=ty.offset + lr * WP + 2 + dx,
                                ap=[ty.ap[0], [GROWS * WP, 2], [WP, RT],
                                    [1, W]])
                            nc.tensor.matmul(
                                pq[o0], lhsT, rhs, start=(tap == 0),
                                stop=(tap == 8), perf_mode=DR)
                    nrm = qn if stage_pfx == "q" else kn
                    for o0, on in OC:
                        qs = io.tile([on, 512], FP16, name=f"{stage_pfx}s{o0}",
                                     tag=f"{stage_pfx}s{o0}", bufs=2)
                        if stage_pfx == "q":
                            nc.scalar.copy(qs, pq[o0])
                        else:
                            nc.vector.tensor_copy(qs, pq[o0])
                        scr = io.tile([on, 512], FP16, name="sqscr",
                                      tag="sqscr", bufs=1)
                        nc.scalar.activation(
                            scr, qs, mybir.ActivationFunctionType.Square,
                            accum_out=nrm[o0][:, t:t + 1])
                        ptr = ps.tile([128, RT * on], FP16,
                                      name=f"ptr{stage_pfx}{o0}", tag="ptp",
                                      bufs=2)
                        for j in range(RT):
                            nc.tensor.transpose(
                                ptr[:, j * on:(j + 1) * on],
                                qs[:, j * 128:(j + 1) * 128],
                                id16[0:on, 0:on])
                        if stage_pfx == "q":
                            dst = bass.AP(
                                tensor=t_qT.tensor,
                                offset=t_qT.offset + r0 * C + o0,
                                ap=[t_qT.ap[0], [C, RT], [1, on]])
                        else:
                            dst = bass.AP(
                                tensor=kT_g.tensor,
                                offset=kT_g.offset + (r0 - 16 * g) * C + o0,
                                ap=[kT_g.ap[0], [C, RT], [1, on]])
                        nc.vector.tensor_copy(dst, ptr)

                def new_v1(m, o0, on):
                    vt = dwp.tile([on, 18 * VS + 1], FP16, name=f"v1_{o0}",
                                  tag=f"v1_{o0}", bufs=2)
                    v1[(m, o0)] = vt
                    # zero the per-row pad column (incl. trailing pad elem)
                    eap = bass.AP(tensor=vt.tensor, offset=vt.offset,
                                  ap=[vt.ap[0], [VS, 19], [1, 1]])
                    nc.vector.memset(eap, 0.0)
                    if m == 0:
                        nc.vector.memset(vt[:, 1:VS], 0.0)
                    if m == NG - 1:
                        nc.vector.memset(vt[:, 17 * VS + 1:18 * VS], 0.0)
                    return vt

                def vconv_tile(t):
                    r0 = RT * t
                    m = t // 4
                    tx = {}
                    for c0, cn in CC:
                        tx[c0] = io.tile([cn, 512], FP16, name=f"xg{c0}",
                                         tag=f"xg{c0}", bufs=2)
                        nc.sync.dma_start(
                            tx[c0], d_x[c0:c0 + cn, 512 * t:512 * (t + 1)])
                    psc = state["psc"]
                    pv = {0: psc.tile([128, 512], F32, name="pv0", tag="pv0",
                                      bufs=1),
                          128: psc.tile([64, 512], F32, name="pv1", tag="pv1",
                                        bufs=1)}
                    for o0, on in OC:
                        for i, (c0, cn) in enumerate(CC):
                            nc.tensor.matmul(
                                pv[o0], t_wv[c0][:, o0:o0 + on], tx[c0],
                                start=(i == 0), stop=(i == 1))
                    lo = r0 - 16 * m + 1
                    for o0, on in OC:
                        vt = v1.get((m, o0))
                        if vt is None:
                            vt = new_v1(m, o0, on)
                        dst = bass.AP(tensor=vt.tensor,
                                      offset=vt.offset + lo * VS + 1,
                                      ap=[vt.ap[0], [VS, RT], [1, W]])
                        src = pv[o0][:, :].rearrange("p (a b) -> p a b", b=W)
                        nc.scalar.copy(dst, src)
                        if r0 % 16 == 0 and m >= 1:
                            pvt = v1[(m - 1, o0)]
                            nc.scalar.copy(pvt[:, 17 * VS + 1:18 * VS],
                                           pv[o0][:, 0:W])
                        if (r0 + 3) % 16 == 15 and m + 1 <= NG - 1:
                            nxt = v1.get((m + 1, o0))
                            if nxt is None:
                                nxt = new_v1(m + 1, o0, on)
                            nc.scalar.copy(nxt[:, 1:VS],
                                           pv[o0][:, 3 * W:4 * W])

                def vstencil_group(g):
                    for o0, on in OC:
                        src = v1[(g, o0)]
                        wcol = t_wdw[o0]
                        out = v_sb[o0]
                        acc = bass.AP(
                            tensor=out.tensor,
                            offset=out.offset + 2048 * g,
                            ap=[out.ap[0], [W, 16], [1, W]])

                        def win(tap):
                            dy, dx = TAPS[tap]
                            return bass.AP(
                                tensor=src.tensor,
                                offset=src.offset + (1 + dy) * VS + 1 + dx,
                                ap=[src.ap[0], [VS, 16], [1, W]])

                        nc.vector.tensor_scalar_mul(acc, win(4), wcol[:, 4:5])
                        for (ta, tb), eng in LEAF_PAIRS:
                            za = dwp.tile([on, 16 * W], FP16, name="dwza",
                                          tag="dwz", bufs=3)
                            zb = dwp.tile([on, 16 * W], FP16, name="dwzb",
                                          tag="dwz", bufs=3)
                            for tap, zt in ((ta, za), (tb, zb)):
                                z = zt[:, :].rearrange("p (a b) -> p a b",
                                                       b=W)
                                if MUL_ENG[tap] == 'a':
                                    nc.scalar.mul(z, win(tap),
                                                  wcol[:, tap:tap + 1])
                                else:
                                    nc.vector.tensor_scalar_mul(
                                        z, win(tap), wcol[:, tap:tap + 1])
                            if eng == 'p':
                                nc.gpsimd.tensor_add(za, za, zb)
                            else:
                                nc.vector.tensor_add(za, za, zb)
                            zv = za[:, :].rearrange("p (a b) -> p a b", b=W)
                            nc.vector.tensor_add(acc, acc, zv)

                def gram_group(g, kT_g):
                    for j in range(GR):
                        r = 16 * g + j
                        for c0, cn in OC:
                            nc.tensor.matmul(
                                pG[c0], qT3[:, r, c0:c0 + cn],
                                kT_g[:, j * C:(j + 1) * C],
                                start=(r == 0 and c0 == 0),
                                stop=(r == H - 1 and c0 == 128))

                with tc.tile_pool(name="psc", bufs=1, space="PSUM") as psc:
                    state["psc"] = psc
                    for g in range(NG):
                        kT_g = io.tile([128, GR * C], FP16, name="kTg",
                                       tag="kTg", bufs=1)
                        for t in range(4 * g, 4 * g + 4):
                            if "q" in parts:
                                fp8_conv_tile(t, d_y8, t_wq8, ("pq0", "pq1"),
                                              "q")
                            if "k" in parts:
                                fp8_conv_tile(t, d_x8, t_wk8, ("pk0", "pk1"),
                                              "k", kT_g=kT_g)
                            if "v" in parts:
                                vconv_tile(t)
                        if "gram" in parts:
                            gram_group(g, kT_g)
                        if g >= 1 and "dw" in parts:
                            vstencil_group(g - 1)
                    if "dw" in parts:
                        vstencil_group(NG - 1)
                if "attn" not in parts:
                    return

                # ---------------- attention ----------------
                rq = {}
                for c0, cn in OC:
                    s = sm.tile([cn, 1], F32, name=f"qn2_{c0}",
                                tag=f"qn2_{c0}")
                    nc.vector.tensor_reduce(s, qn[c0],
                                            axis=mybir.AxisListType.X,
                                            op=mybir.AluOpType.add)
                    nc.scalar.sqrt(s, s)
                    nc.vector.reciprocal(s, s)
                    nc.vector.tensor_mul(s, s, t_tv[c0][:, 0:1])
                    rq[c0] = s
                kn2 = {}
                for c0, cn in OC:
                    s = sm.tile([cn, 1], F32, name=f"kn2_{c0}",
                                tag=f"kn2_{c0}")
                    nc.vector.tensor_reduce(s, kn[c0],
                                            axis=mybir.AxisListType.X,
                                            op=mybir.AluOpType.add)
                    kn2[c0] = s
                psf_cm = tc.tile_pool(name="psf", bufs=1, space="PSUM")
                psf = psf_cm.__enter__()
                pkrow = psf.tile([1, C], F32, name="pkrow", tag="pfsm",
                                 bufs=1)
                nc.tensor.transpose(pkrow[0:1, 0:128], kn2[0], id32)
                nc.tensor.transpose(pkrow[0:1, 128:192], kn2[128],
                                    id32[0:64, 0:64])
                krow = sm.tile([1, C], F32, name="krow", tag="krow")
                nc.scalar.copy(krow, pkrow)
                nc.scalar.sqrt(krow, krow)
                nc.vector.reciprocal(krow, krow)
                rkb = {}
                for c0, cn in OC:
                    pb = psf.tile([cn, C], F32, name=f"prkb{c0}", tag="pfsm",
                                  bufs=1)
                    nc.tensor.matmul(pb, ones1[0:1, c0:c0 + cn], krow,
                                     start=True, stop=True)
                    sb_ = sm.tile([cn, C], F32, name=f"rkb{c0}",
                                  tag=f"rkb{c0}")
                    nc.scalar.copy(sb_, pb)
                    rkb[c0] = sb_
                A = {}
                for c0, cn in OC:
                    s = sm.tile([cn, C], F32, name=f"S{c0}", tag=f"S{c0}")
                    nc.vector.tensor_scalar_mul(s, pG[c0], rq[c0])
                    nc.vector.tensor_mul(s, s, rkb[c0])
                    nc.vector.tensor_add(s, s, t_msk[c0])
                    m = sm.tile([cn, 1], F32, name=f"m{c0}", tag=f"m{c0}")
                    a = sm.tile([cn, C], FP16, name=f"A{c0}", tag=f"A{c0}")
                    z = sm.tile([cn, 1], F32, name=f"z{c0}", tag=f"z{c0}")
                    nc.vector.tensor_reduce(
                        m, s, axis=mybir.AxisListType.X,
                        op=mybir.AluOpType.max)
                    nc.vector.tensor_scalar_mul(m, m, -1.0)
                    nc.scalar.activation(
                        a, s, mybir.ActivationFunctionType.Exp,
                        bias=m, scale=1.0, accum_out=z)
                    nc.vector.reciprocal(z, z)
                    nc.vector.tensor_scalar_mul(a, a, z)
                    A[c0] = a
                CT = {}
                for d0, dn in OC:
                    pc = psf.tile([dn, C], F32, name=f"pCT{d0}", tag="pfsm",
                                  bufs=1)
                    for i, (c0, cn) in enumerate(OC):
                        nc.tensor.matmul(
                            pc, A[c0][:, d0:d0 + dn], t_wp_[c0],
                            start=(i == 0), stop=(i == 1))
                    ct_ = sm.tile([dn, C], FP16, name=f"CT{d0}",
                                  tag=f"CT{d0}")
                    nc.scalar.copy(ct_, pc)
                    CT[d0] = ct_

                # ---------------- final conv ----------------
                ostb = {}
                for t in range(NT):
                    if t % 2 == 0:
                        ostb = {o0: io.tile([on, 1024], FP16,
                                            name=f"ost{o0}", tag=f"ost{o0}",
                                            bufs=2)
                                for o0, on in OC}
                    for o0, on in OC:
                        pf = psf.tile([on, 512], F32, name=f"pf{o0}",
                                      tag=f"pf{o0}", bufs=2)
                        for i, (d0, dn) in enumerate(OC):
                            nc.tensor.matmul(
                                pf, CT[d0][:, o0:o0 + on],
                                v_sb[d0][:, 512 * t:512 * (t + 1)],
                                start=(i == 0), stop=(i == 1))
                        sl = ostb[o0][:, 512 * (t % 2):512 * (t % 2 + 1)]
                        if o0 == 0:
                            nc.scalar.copy(sl, pf)
                        else:
                            nc.vector.tensor_copy(sl, pf)
                    if t % 2 == 1:
                        for o0, on in OC:
                            nc.sync.dma_start(
                                d_out[o0:o0 + on,
                                      512 * (t - 1):512 * (t + 1)],
                                ostb[o0])
                psf_cm.__exit__(None, None, None)

            if use_for_i and repeat > 1:
                with tc.For_i(0, repeat, 1) as iv:
                    body(iv)
            else:
                for it in range(repeat):
                    body(it)

            o16 = sm.tile([128, 16], F32, name="o16", tag="o16")
            nc.vector.tensor_copy(o16, tki)
            nc.sync.dma_start(d_tok[:, :], o16)

    nc.compile()
    return nc


# ---------------------------------------------------------------------------
# host-side packing
# ---------------------------------------------------------------------------

def _fold8(w_tap):
    """w_tap[o, i, tap] float32 -> fp8 lhsT layout [96, 9*2*C] with
    per-output-channel scaling into fp8 range."""
    s_o = 224.0 / np.abs(w_tap).max(axis=(1, 2))
    ws = w_tap * s_o[:, None, None]
    w8 = np.zeros((96, 9 * 2 * C), np.float32)
    for tap in range(9):
        for j in range(2):
            w8[:, tap * 2 * C + j * C:tap * 2 * C + (j + 1) * C] = \
                ws[:, 96 * j:96 * (j + 1), tap].T
    return w8.astype(mybir.dt.np(FP8))


def prep_weights(kv_w, kv_dw_w, q_w, q_dw_w, proj_w, temperature):
    kv_w = np.asarray(kv_w, np.float32).reshape(2 * C, C)
    kv_dw_w = np.asarray(kv_dw_w, np.float32).reshape(2 * C, 9)
    q_w = np.asarray(q_w, np.float32).reshape(C, C)
    q_dw_w = np.asarray(q_dw_w, np.float32).reshape(C, C, 9)
    proj_w = np.asarray(proj_w, np.float32).reshape(C, C)
    temperature = np.asarray(temperature, np.float32).reshape(HEADS)

    wq8 = _fold8(np.einsum('oct,ci->oit', q_dw_w, q_w))
    # k branch: W_tap[d, i, tap] = kdw[d, tap] * Wk[d, i]
    wk_tap = kv_dw_w[:C, None, :] * kv_w[:C, :, None]
    wk8 = _fold8(wk_tap)
    wv_lhsT = np.ascontiguousarray(kv_w[C:].T)               # [ci, vout]
    wdwv = np.zeros((C, 16), np.float32)
    wdwv[:, :9] = kv_dw_w[C:]
    wproj_T = np.ascontiguousarray(proj_w.T)                 # [c, o]
    tvec = np.zeros((C, 16), np.float32)
    tvec[:, 0] = np.repeat(temperature, CH)
    smask = np.full((C, C), -60000.0, np.float32)
    for h in range(HEADS):
        smask[h * CH:(h + 1) * CH, h * CH:(h + 1) * CH] = 0.0
    return (wq8, wk8, wv_lhsT.astype(np.float16), wdwv,
            wproj_T.astype(np.float16), tvec, smask)


def _pad8(plane):
    """[C, H, W] fp32 -> [96, 2*PL] fp8 padded 2-block layout."""
    np8 = mybir.dt.np(FP8)
    yp = np.zeros((2, 96, HP + 2, WP), np8)
    p8 = plane.astype(np8)
    yp[0, :, 2:2 + H, 2:2 + W] = p8[0:96]
    yp[1, :, 2:2 + H, 2:2 + W] = p8[96:192]
    return np.ascontiguousarray(yp.transpose(1, 0, 2, 3)).reshape(96, 2 * PL)


def prep_image(xi, yi):
    xi = np.asarray(xi, np.float32).reshape(C, H, W)
    yi = np.asarray(yi, np.float32).reshape(C, H, W)
    x16 = xi.reshape(C, N).astype(np.float16)
    return x16, _pad8(xi), _pad8(yi)


def make_in_maps(x, y, kv_w, kv_dw_w, q_w, q_dw_w, proj_w, temperature):
    x = np.asarray(x, np.float32)
    y = np.asarray(y, np.float32)
    wq8, wk8, wv, wdwv, wpj, tv, smask = prep_weights(
        kv_w, kv_dw_w, q_w, q_dw_w, proj_w, temperature)
    tok = np.zeros((128, 16), np.float32)
    in_maps = []
    for b in range(B):
        x16, x8, y8 = prep_image(x[b], y[b])
        in_maps.append({
            "tok_in": tok, "ypad8": y8, "xpad8": x8, "x16": x16,
            "wq8": wq8, "wk8": wk8, "wv": wv, "wdwv": wdwv,
            "wproj": wpj, "tvec": tv, "smask": smask,
        })
    return in_maps


_CACHE = {}


def kernel(x, y, kv_w, kv_dw_w, q_w, q_dw_w, proj_w, temperature):
    in_maps = make_in_maps(x, y, kv_w, kv_dw_w, q_w, q_dw_w, proj_w,
                           temperature)
    if "nc" not in _CACHE:
        _CACHE["nc"] = build()
    nc = _CACHE["nc"]
    from concourse.bass_utils import run_bass_kernel_spmd
    res = run_bass_kernel_spmd(nc, in_maps, core_ids=list(range(B)))
    out = np.stack([res.results[b]["out"].reshape(C, H, W) for b in range(B)])
    return out.astype(np.float32)
